# revision 34
# baseline (speedup 1.0000x reference)
"""Trainium2 Bass kernel for CachedMixtralAttention (sliding-window GQA attention).

Strategy (8 NeuronCores, tensor-parallel over KV-head groups):
  - Core i handles KV head i and its 4 query heads (GQA group). Wq/Wk/Wv are
    sliced on the head axis, Wo on the input-head axis. Each core computes a
    partial output [S, HID] in bf16; the host sums the 8 partials in fp32.
  - On-device layout is "T layout": QT/KT = [head_dim, seq] so the attention
    contraction dims always sit on SBUF partitions.
  - Softmax skips the max-subtraction (scores ~ N(0,1) after 1/sqrt(d): exp is
    safe in fp32) and applies the mask as a 0/1 multiply after exp, which is
    exactly equivalent to the reference's -1e9 masking.
  - Attention blocks are RAGGED: for each 128-wide k-tile only the 128-aligned
    q-subrange of the chunk that actually attends (<=640-wide sliding-window
    band) is computed, cutting score/AV streaming ~38%. The per-head softmax
    accumulator uses PSUM has_written semantics to accumulate the ragged
    slices correctly.
  - Engine assignment tuned so the PE never waits on a slow serial chain:
      exp                      -> Scalar (ACT), reading score PSUM directly
      mask multiply, esum      -> Vector (bf16, 2x mode)
      denominator              -> PE ones-matmul (esum + last e tile, so the
                                  denominator never waits on the DVE backlog)
      1/den                    -> vector.reciprocal_approx_fast
      PSUM drains              -> split Scalar/Vector
  - Phase 1 runs each query chunk in two passes (K/V projections, then Q) so
    chunk boundaries never stall on PSUM banks; hT is chunk-resident, wq
    streams per chunk on the scalar queue (SBUF is the binding constraint).
  - Out-projection groups are spread one-per-attention-step across the next
    chunk's stream so their PSUM drains (the only fp32->bf16 copies left)
    never burst; output DMAs alternate the sync/gpsimd queues; the DRAM
    store is bf16 and the host sums partials in fp32.
  - A short dummy-matmul warm-up keeps the HAM clock gate at 8/8 while the
    first weights stream in.
"""

from contextlib import ExitStack

import ml_dtypes
import numpy as np

S = 2048
HID = 4096
NUM_Q_HEADS = 32
NUM_KV_HEADS = 8
D = 128                      # head dim
NCORES = 8
HQ = NUM_Q_HEADS // NUM_KV_HEADS  # q heads per core (GQA group size)
QC = 512                     # query chunk (matmul moving free dim)
MAX_WAVELENGTH = 10000.0
INV_NORM = 1.0 / np.sqrt(D)

BF16 = ml_dtypes.bfloat16

# tuning knobs
ESUM_ON_GPSIMD = False       # gpsimd esum steals the shared DVE SBUF port
OPROJ_BF16_PSUM = False      # bf16 PSUM matmul out unsupported in this bass
LOOK = 3                     # score-matmul lookahead depth in attention


def _rope_tables(s):
    """cos/sin tables in T layout [128, s], sign folded into sin. bf16."""
    pos = np.arange(s, dtype=np.float32)
    invf = 1.0 / (MAX_WAVELENGTH ** (np.arange(0, D, 2, dtype=np.float32) / D))
    freq = invf[:, None] * pos[None, :]              # [64, s]
    cosT = np.concatenate([np.cos(freq), np.cos(freq)], axis=0)   # [128, s]
    sinT = np.concatenate([-np.sin(freq), np.sin(freq)], axis=0)  # [128, s]
    return cosT.astype(BF16), sinT.astype(BF16)


def _classify_mask(mask2d, s):
    """Classify ragged [128k x L] blocks of the mask.

    For each (q-chunk c, k-tile g) with any attention, restrict to the
    128-aligned q-subrange [off, off+L) of the chunk that actually attends
    (the sliding window gives each k-tile a ~640-wide q-band, so most blocks
    are narrower than QC — this cuts score/AV matmul streaming by ~38%).

    Returns (blocks, mask_buf): blocks[c] = list of (g, off, L, moff) with
    moff the column offset of the [128, L] mask slice in mask_buf (None if
    the block is all-keep); mask_buf is [128, total] float32.
    """
    mT = np.ascontiguousarray(mask2d.T)  # [k, q]
    n_chunks = s // QC
    n_ktiles = s // 128
    blocks = []
    cols = []
    tile_ids = {}
    total = 0
    for c in range(n_chunks):
        lst = []
        for g in range(n_ktiles):
            blk = mT[g * 128:(g + 1) * 128, c * QC:(c + 1) * QC]
            qs = np.flatnonzero(blk.any(axis=0))
            if qs.size == 0:
                continue
            off = (qs[0] // 128) * 128
            end = min(QC, ((qs[-1] // 128) + 1) * 128)
            sub = blk[:, off:end]
            if sub.all():
                lst.append((g, off, end - off, None))
            else:
                key = sub.tobytes()
                if key not in tile_ids:
                    tile_ids[key] = total
                    cols.append(sub.astype(np.float32))
                    total += sub.shape[1]
                lst.append((g, off, end - off, tile_ids[key]))
        assert lst, f"query chunk {c} attends to nothing"
        blocks.append(lst)
    if not cols:
        cols.append(np.zeros((128, 128), np.float32))
        total = 128
    return blocks, np.concatenate(cols, axis=1)


def _build_program(s, hid, blocks, mask_cols):
    """Emit the Bass/Tile program. Same program runs SPMD on all 8 cores."""
    import concourse.bacc as bacc
    import concourse.mybir as mybir
    import concourse.tile as tile
    from concourse import bass_isa

    dt = mybir.dt
    HT = hid // 128          # hidden contraction tiles (32)
    C = s // QC              # query chunks (4)
    PT = 4                   # wq piece size in t-tiles
    NP = HT // PT            # wq pieces per chunk (8)
    HB = 4                   # hid tiles per hT DMA batch

    nc = bacc.Bacc("TRN2", target_bir_lowering=False, debug=False,
                   num_devices=NCORES)

    # inputs are host-prepacked into SBUF-image layouts (partition-major) so
    # every DMA moves multi-KB contiguous runs per partition
    hT_d = nc.declare_dram_parameter("hT", [128, HT * s], dt.bfloat16, isOutput=False)
    wq_d = nc.declare_dram_parameter("wq", [128, HT * HQ * D], dt.bfloat16, isOutput=False)
    wk_d = nc.declare_dram_parameter("wk", [128, hid], dt.bfloat16, isOutput=False)
    wv_d = nc.declare_dram_parameter("wv", [128, hid], dt.bfloat16, isOutput=False)
    wo_d = nc.declare_dram_parameter("wo", [128, HQ * hid], dt.bfloat16, isOutput=False)
    cos_d = nc.declare_dram_parameter("cosT", [128, s], dt.bfloat16, isOutput=False)
    sin_d = nc.declare_dram_parameter("sinT", [128, s], dt.bfloat16, isOutput=False)
    msk_d = nc.declare_dram_parameter("masks", [128, mask_cols], dt.bfloat16, isOutput=False)
    eye_d = nc.declare_dram_parameter("eye", [128, 256], dt.bfloat16, isOutput=False)
    out_d = nc.declare_dram_parameter("out", [s, hid], dt.bfloat16, isOutput=True)

    with ExitStack() as ctx:
        tc = ctx.enter_context(tile.TileContext(nc))
        const = ctx.enter_context(tc.tile_pool(name="const", bufs=1))
        hpool = ctx.enter_context(tc.tile_pool(name="hpool", bufs=3))
        wqpool = ctx.enter_context(tc.tile_pool(name="wqpool", bufs=3))
        epool = ctx.enter_context(tc.tile_pool(name="epool", bufs=2))
        tpool = ctx.enter_context(tc.tile_pool(name="tpool", bufs=3))
        opool = ctx.enter_context(tc.tile_pool(name="opool", bufs=8))
        psum = ctx.enter_context(tc.tile_pool(name="psum", bufs=6, space="PSUM"))
        # at_ps accumulates across a whole head; in the shared rotation it
        # would stall the next head's first score alloc, so it gets own banks
        # at_ps + den alternate through this 2-bank pool; both are short-
        # lived relative to the main rotation and would stall sc allocs there
        psum_at = ctx.enter_context(tc.tile_pool(name="psum_at", bufs=2,
                                                 space="PSUM"))

        # ---- one-time loads ----
        # wk/wv resident (1MB each), wo resident (4.2MB, loaded after start),
        # wq streamed per chunk in pieces. hT double-buffered per chunk.
        wk_sb = const.tile([128, HT * D], dt.bfloat16, tag="wk")
        wv_sb = const.tile([128, HT * D], dt.bfloat16, tag="wv")
        eye_sb = const.tile([128, 256], dt.bfloat16, tag="eye")
        nc.gpsimd.dma_start(eye_sb[:], eye_d[:])  # [eye | ones]
        cos_sb = const.tile([128, s], dt.bfloat16, tag="cos")
        sin_sb = const.tile([128, s], dt.bfloat16, tag="sin")
        nc.gpsimd.dma_start(cos_sb[:], cos_d[:])
        nc.gpsimd.dma_start(sin_sb[:], sin_d[:])
        msk_sb = const.tile([128, mask_cols], dt.bfloat16, tag="msk")
        wo_sb = const.tile([128, HQ * hid], dt.bfloat16, tag="wo")

        # warm-up burst: dummy matmuls on a memset scratch keep the PE busy
        # while the first weights/activations stream in, so the HAM clock
        # gate reaches 8/8 before real work starts (else chunk 0 runs 1.2GHz)
        warm_sb = const.tile([128, QC], dt.bfloat16, tag="warm")
        nc.vector.memset(warm_sb[:], 0.0)
        warm_ps = psum.tile([128, QC], dt.float32, tag="ps", name="warm")
        for wi in range(16):
            nc.tensor.matmul(warm_ps[:], warm_sb[:, 0:128], warm_sb[:],
                             start=(wi == 0), stop=(wi == 15))

        # persistent per-chunk tensors
        q_sb = [[const.tile([128, QC], dt.bfloat16, tag=f"q{c}_{h}", name=f"q{c}_{h}")
                 for h in range(HQ)] for c in range(C)]
        kt_sb = [const.tile([128, QC], dt.bfloat16, tag=f"kt{c}", name=f"kt{c}")
                 for c in range(C)]
        v_sb = [[const.tile([128, 128], dt.bfloat16, tag=f"v{c}_{j}", name=f"v{c}_{j}")
                 for j in range(QC // 128)] for c in range(C)]
        atpool = ctx.enter_context(tc.tile_pool(name="atpool", bufs=8))
        at_t = {}            # (c, h) -> normalized attnT tile (rotating pool)

        # ---- phase 1: QKV projections (T layout) + RoPE + V transpose ----
        # Per chunk: pass A accumulates K/V (2 PSUM banks), pass B the 4 Q
        # heads (4 banks). hT chunk is SBUF-resident across both passes, so
        # only ~6 banks are ever live and boundaries never stall the PE.
        def rope_math(a, dest, c):
            # dest = a * cos + swap_halves(a) * sin   (all bf16, DVE 2x mode)
            cosc = cos_sb[:, c * QC:(c + 1) * QC]
            sinc = sin_sb[:, c * QC:(c + 1) * QC]
            b = tpool.tile([128, QC], dt.bfloat16, bufs=2, name="b")
            nc.gpsimd.dma_start(b[0:64, :], a[64:128, :])
            nc.gpsimd.dma_start(b[64:128, :], a[0:64, :])
            t1 = tpool.tile([128, QC], dt.bfloat16, bufs=2, name="t1")
            nc.vector.tensor_mul(t1[:], a[:], cosc)
            nc.vector.tensor_mul(b[:], b[:], sinc)
            nc.vector.tensor_add(dest[:], t1[:], b[:])

        for c in range(C):
            if c == 1 % C:
                nc.gpsimd.dma_start(msk_sb[:], msk_d[:])
            if c == 2 % C:
                for p in range(4):
                    q4 = HQ * hid // 4
                    nc.gpsimd.dma_start(wo_sb[:, p * q4:(p + 1) * q4],
                                        wo_d[:, p * q4:(p + 1) * q4])
            # hT chunk load: two half-chunk tiles (ring of 3), 4 batch DMAs
            # each. On chunk 0 the wk/wv pieces interleave with the hT batches
            # in PE consumption order so pass A never waits on a late weight.
            HH = HT // 2
            WP = HT // 8 * D
            halves = []
            for hf in range(2):
                htile = hpool.tile([128, HH * QC], dt.bfloat16, name="htc")
                for tb in range(HH // HB):
                    if c == 0:
                        p = hf * (HH // HB) + tb
                        nc.sync.dma_start(wk_sb[:, p * WP:(p + 1) * WP],
                                          wk_d[:, p * WP:(p + 1) * WP])
                        nc.sync.dma_start(wv_sb[:, p * WP:(p + 1) * WP],
                                          wv_d[:, p * WP:(p + 1) * WP])
                    base = (c * HT + hf * HH + tb * HB) * QC
                    nc.sync.dma_start(htile[:, tb * HB * QC:(tb + 1) * HB * QC],
                                      hT_d[:, base:base + HB * QC])
                halves.append(htile)

            def ht_at(t):
                return halves[t // HH][:, (t % HH) * QC:(t % HH + 1) * QC]
            # wq pieces for this chunk stream on the sync queue; piece i is
            # prefetched while piece i-1 computes (pass B below)
            wq_pc = []
            wq_q = nc.scalar
            for p in range(NP):
                wt = wqpool.tile([128, PT * HQ * D], dt.bfloat16, name="wqp")
                wq_q.dma_start(wt[:], wq_d[:, p * PT * HQ * D:(p + 1) * PT * HQ * D])
                wq_pc.append(wt)

            # pass A: K/V accumulation
            kt_ps = psum.tile([128, QC], dt.float32, tag="ps")
            vt_ps = psum.tile([128, QC], dt.float32, tag="ps")
            for t in range(HT):
                ht = ht_at(t)
                st, sp = (t == 0), (t == HT - 1)
                nc.tensor.matmul(kt_ps[:], wk_sb[:, t * D:(t + 1) * D], ht,
                                 start=st, stop=sp)
                nc.tensor.matmul(vt_ps[:], wv_sb[:, t * D:(t + 1) * D], ht,
                                 start=st, stop=sp)

            # drain K/V on scalar engine (fp32 PSUM -> bf16 SBUF)
            ka = tpool.tile([128, QC], dt.bfloat16, bufs=2, name="ka")
            nc.scalar.copy(ka[:], kt_ps[:])
            vtT = epool.tile([128, QC], dt.bfloat16, bufs=2, name="vtT")
            nc.scalar.copy(vtT[:], vt_ps[:])

            # pass B: 4 Q heads
            qt_ps = [psum.tile([128, QC], dt.float32, name=f"qt_ps{h}", tag="ps")
                     for h in range(HQ)]
            for t in range(HT):
                p, ts_ = t // PT, t % PT
                ht = ht_at(t)
                st, sp = (t == 0), (t == HT - 1)
                wt = wq_pc[p]
                for h in range(HQ):
                    nc.tensor.matmul(qt_ps[h][:],
                                     wt[:, (ts_ * HQ + h) * D:(ts_ * HQ + h + 1) * D],
                                     ht, start=st, stop=sp)

                # K rope + V transpose overlap the early Q accumulation
                if t == 0:
                    rope_math(ka, kt_sb[c], c)
                if t == 2:
                    for j in range(QC // 128):
                        tp = psum.tile([128, 128], dt.bfloat16, tag="ps")
                        nc.tensor.transpose(tp[:], vtT[:, j * 128:(j + 1) * 128],
                                            eye_sb[:, 0:128])
                        nc.vector.tensor_copy(v_sb[c][j][:], tp[:])
            for h in range(HQ):
                qa = tpool.tile([128, QC], dt.bfloat16, bufs=2, name=f"qa{h}")
                nc.scalar.copy(qa[:], qt_ps[h][:])
                rope_math(qa, q_sb[c][h], c)

        # ---- phase 2: attention per (chunk, head) ----
        # Score matmuls run LOOK blocks ahead of the AV matmuls so the PE
        # never waits on the exp/mask chain; normalization of head h is
        # emitted during head h+1's stream, out-proj of chunk c during c+1.
        ones_sb = eye_sb[:, 128:256]  # [128,128] ones

        def emit_score(c, h, g, off, L, moff, ebuf, bi):
            # ragged block: only the 128-aligned q-subrange [off, off+L) of
            # the chunk attends to k-tile g (sliding window band)
            kc, j = g // (QC // 128), g % (QC // 128)
            sc = psum.tile([128, QC], dt.float32, tag="ps", name="sc")
            nc.tensor.matmul(sc[:, 0:L], kt_sb[kc][:, j * 128:(j + 1) * 128],
                             q_sb[c][h][:, off:off + L], start=True, stop=True)
            e = ebuf[:, bi * QC:bi * QC + L]
            nc.scalar.activation(e, sc[:, 0:L],
                                 mybir.ActivationFunctionType.Exp,
                                 scale=float(INV_NORM))
            if moff is not None:
                nc.vector.tensor_mul(e, e, msk_sb[:, moff:moff + L])
            return e

        def emit_norm(c, h, atu, esum, e_last, off_l, L_l):
            # den = ones^T @ (esum + e_last): partition-sum via two PE matmuls.
            # e_last skips the esum chain so den never waits on the DVE backlog
            den_ps = psum.tile([128, QC], dt.float32, tag="ps", name="den")
            nc.tensor.matmul(den_ps[:], ones_sb, esum[:], start=True, stop=False)
            nc.tensor.matmul(den_ps[:, off_l:off_l + L_l], ones_sb, e_last,
                             start=False, stop=True)
            recf = tpool.tile([128, QC], dt.float32, tag="recf", bufs=2, name="recf")
            nc.vector.reciprocal_approx_fast(out=recf[:], in_=den_ps[:])
            att = atpool.tile([128, QC], dt.bfloat16, name="at")
            nc.vector.tensor_mul(att[:], atu[:], recf[:])
            at_t[(c, h)] = att

        OC = 1024 if OPROJ_BF16_PSUM else QC
        o_dt = dt.bfloat16 if OPROJ_BF16_PSUM else dt.float32

        oq = []              # pending out-proj groups, drained 1 per step

        def emit_ogroup(c, oc, r):
            o_ps = psum.tile([128, OC], o_dt, tag="ps", name="o_ps")
            for h in range(HQ):
                nc.tensor.matmul(o_ps[:],
                                 at_t[(c, h)][:, r * 128:(r + 1) * 128],
                                 wo_sb[:, h * hid + oc * OC: h * hid + (oc + 1) * OC],
                                 start=(h == 0), stop=(h == HQ - 1))
            ob = opool.tile([128, OC], dt.bfloat16, name="ob")
            row = c * QC + r * 128
            if (oc + r) % 2 == 0:
                nc.vector.tensor_copy(ob[:], o_ps[:])
            else:
                nc.scalar.copy(ob[:], o_ps[:])
            dma_q = (nc.sync, nc.gpsimd)[(oc * 4 + r) % 2]
            dma_q.dma_start(out_d[row:row + 128, oc * OC:(oc + 1) * OC], ob[:])

        def emit_outproj(c):
            for oc in range(hid // OC):
                for r in range(QC // 128):
                    oq.append((c, oc, r))

        esum_eng = nc.gpsimd if ESUM_ON_GPSIMD else nc.vector

        stream = []          # (c, h, i, n, g, off, L, moff)
        for c in range(C):
            for h in range(HQ):
                blks = blocks[c]
                for i, (g, off, L, moff) in enumerate(blks):
                    stream.append((c, h, i, len(blks), g, off, L, moff))
        N = len(stream)
        es = {}
        ebufs = {}
        pend = None          # (c, h, atu, esum, e_last, off, L) pending norm
        oc_pend = None       # (c, h) whose norm just fired; outproj pends
        pf = 0               # prefetch pointer

        at_ps = None
        esum = None
        for i in range(N):
            c, h, bi, n, g, off, L, moff = stream[i]
            while pf < N and pf < i + LOOK + 1:
                if pf <= i:
                    pf = i
                cc, hh, bb, nn, gg, oo, ll, mm = stream[pf]
                if bb == 0:
                    ebufs[(cc, hh)] = epool.tile([128, 8 * QC], dt.bfloat16,
                                                 name="eb")
                es[pf] = emit_score(cc, hh, gg, oo, ll, mm,
                                    ebufs[(cc, hh)], bb)
                pf += 1
            if bi == 0:
                at_ps = psum_at.tile([128, QC], dt.float32, tag="at", name="at_ps")
                esum = tpool.tile([128, QC], dt.bfloat16, tag="esum", bufs=2,
                                  name="esum")
                nc.vector.memset(esum[:], 0.0)
            kc, j = g // (QC // 128), g % (QC // 128)
            st, sp = (bi == 0), (bi == n - 1)
            e = es.pop(i)
            nc.tensor.matmul(at_ps[:, off:off + L], v_sb[kc][j][:], e,
                             start=st, stop=sp)
            if bi < n - 1:
                esum_eng.tensor_add(esum[:, off:off + L],
                                    esum[:, off:off + L], e)
            if oq:
                emit_ogroup(*oq.pop(0))
            if bi == min(2, n - 1) and pend is not None:
                emit_norm(*pend)
                oc_pend = (pend[0], pend[1])
                pend = None
            if bi == min(6, n - 2) and oc_pend is not None:
                if oc_pend[1] == HQ - 1:
                    emit_outproj(oc_pend[0])
                oc_pend = None
            if bi == n - 1:
                # drain the unnormalized attnT on the scalar engine
                atu = tpool.tile([128, QC], dt.bfloat16, tag="atu", bufs=2,
                                 name="atu")
                nc.scalar.copy(atu[:], at_ps[:])
                ebufs.pop((c, h), None)
                assert pend is None
                pend = (c, h, atu, esum, e, off, L)
        emit_norm(*pend)
        emit_outproj(C - 1)
        while oq:
            emit_ogroup(*oq.pop(0))

    nc.compile()
    return nc


def _prep_inputs(hidden_states, attention_mask, Wq, Wk, Wv, Wo):
    """Host-side sharding + layout prep. Returns (in_maps, blocks, n_mask, s, hid)."""
    hs = np.asarray(hidden_states)
    assert hs.shape[0] == 1, "kernel assumes batch 1"
    s, hid = hs.shape[1], hs.shape[2]
    mask = np.asarray(attention_mask)[0]
    Wq = np.asarray(Wq); Wk = np.asarray(Wk); Wv = np.asarray(Wv); Wo = np.asarray(Wo)

    # SBUF-image packing: x[(t p), c] -> [p, (t c)] so DMAs are contiguous
    def pack(w, tiles):
        return np.ascontiguousarray(
            w.reshape(tiles, 128, -1).transpose(1, 0, 2).reshape(128, -1)
        ).astype(BF16)

    hTn = np.asarray(hs[0].T).reshape(hid // 128, 128, s // QC, QC)
    hT = np.ascontiguousarray(hTn.transpose(1, 2, 0, 3).reshape(128, -1)).astype(BF16)
    # layout: hT[p, ((c * HT + t) * QC + q)]
    cosT, sinT = _rope_tables(s)
    blocks, mask_buf = _classify_mask(mask, s)
    eye_ones = np.concatenate(
        [np.eye(128, dtype=np.float32), np.ones((128, 128), np.float32)],
        axis=1).astype(BF16)

    masks_pk = np.ascontiguousarray(mask_buf).astype(BF16)
    mask_cols = masks_pk.shape[1]

    in_maps = []
    for i in range(NCORES):
        wq_i = pack(Wq[:, i * HQ:(i + 1) * HQ, :].reshape(hid, HQ * D), hid // 128)
        wk_i = pack(Wk[:, i, :], hid // 128)
        wv_i = pack(Wv[:, i, :], hid // 128)
        wo_i = pack(Wo[i * HQ:(i + 1) * HQ].reshape(HQ * D, hid), HQ)
        in_maps.append({
            "hT": hT, "wq": wq_i, "wk": wk_i, "wv": wv_i, "wo": wo_i,
            "cosT": cosT, "sinT": sinT, "masks": masks_pk, "eye": eye_ones,
        })
    return in_maps, blocks, mask_cols, s, hid


def _run(hidden_states, attention_mask, Wq, Wk, Wv, Wo, trace=False):
    from concourse.bass_utils import run_bass_kernel_spmd

    in_maps, blocks, mask_cols, s, hid = _prep_inputs(
        hidden_states, attention_mask, Wq, Wk, Wv, Wo)
    nc = _build_program(s, hid, blocks, mask_cols)
    res = run_bass_kernel_spmd(nc, in_maps, core_ids=list(range(NCORES)),
                               trace=trace)
    out = np.zeros((s, hid), np.float32)
    for i in range(NCORES):
        out += res.results[i]["out"].astype(np.float32)
    return out[None, :, :], res


def kernel(hidden_states, attention_mask, Wq, Wk, Wv, Wo):
    out, _ = _run(hidden_states, attention_mask, Wq, Wk, Wv, Wo, trace=False)
    return out


# revision 35
# speedup vs baseline: 1.0096x; 1.0096x over previous
"""Trainium2 Bass kernel for CachedMixtralAttention (sliding-window GQA attention).

Strategy (8 NeuronCores, tensor-parallel over KV-head groups):
  - Core i handles KV head i and its 4 query heads (GQA group). Wq/Wk/Wv are
    sliced on the head axis, Wo on the input-head axis. Each core computes a
    partial output [S, HID] in bf16; the host sums the 8 partials in fp32.
  - On-device layout is "T layout": QT/KT = [head_dim, seq] so the attention
    contraction dims always sit on SBUF partitions.
  - Softmax skips the max-subtraction (scores ~ N(0,1) after 1/sqrt(d): exp is
    safe in fp32) and applies the mask as a 0/1 multiply after exp, which is
    exactly equivalent to the reference's -1e9 masking.
  - Attention blocks are RAGGED: for each 128-wide k-tile only the 128-aligned
    q-subrange of the chunk that actually attends (<=640-wide sliding-window
    band) is computed, cutting score/AV streaming ~38%. The per-head softmax
    accumulator uses PSUM has_written semantics to accumulate the ragged
    slices correctly.
  - Engine assignment tuned so the PE never waits on a slow serial chain:
      exp                      -> Scalar (ACT), reading score PSUM directly
      mask multiply, esum      -> Vector (bf16, 2x mode)
      denominator              -> PE ones-matmul (esum + last e tile, so the
                                  denominator never waits on the DVE backlog)
      1/den                    -> vector.reciprocal_approx_fast
      PSUM drains              -> split Scalar/Vector
  - Phase 1 runs each query chunk in two passes (K/V projections, then Q) so
    chunk boundaries never stall on PSUM banks; hT is chunk-resident, wq
    streams per chunk on the scalar queue (SBUF is the binding constraint).
  - Out-projection groups are spread one-per-attention-step across the next
    chunk's stream so their PSUM drains (the only fp32->bf16 copies left)
    never burst; output DMAs alternate the sync/gpsimd queues; the DRAM
    store is bf16 and the host sums partials in fp32.
  - A short dummy-matmul warm-up keeps the HAM clock gate at 8/8 while the
    first weights stream in.
"""

from contextlib import ExitStack

import ml_dtypes
import numpy as np

S = 2048
HID = 4096
NUM_Q_HEADS = 32
NUM_KV_HEADS = 8
D = 128                      # head dim
NCORES = 8
HQ = NUM_Q_HEADS // NUM_KV_HEADS  # q heads per core (GQA group size)
QC = 512                     # query chunk (matmul moving free dim)
MAX_WAVELENGTH = 10000.0
INV_NORM = 1.0 / np.sqrt(D)

BF16 = ml_dtypes.bfloat16

# tuning knobs
ESUM_ON_GPSIMD = False       # gpsimd esum steals the shared DVE SBUF port
OPROJ_BF16_PSUM = False      # bf16 PSUM matmul out unsupported in this bass
LOOK = 3                     # score-matmul lookahead depth in attention


def _rope_tables(s):
    """cos/sin tables in T layout [128, s], sign folded into sin. bf16."""
    pos = np.arange(s, dtype=np.float32)
    invf = 1.0 / (MAX_WAVELENGTH ** (np.arange(0, D, 2, dtype=np.float32) / D))
    freq = invf[:, None] * pos[None, :]              # [64, s]
    cosT = np.concatenate([np.cos(freq), np.cos(freq)], axis=0)   # [128, s]
    sinT = np.concatenate([-np.sin(freq), np.sin(freq)], axis=0)  # [128, s]
    return cosT.astype(BF16), sinT.astype(BF16)


def _classify_mask(mask2d, s):
    """Classify ragged [128k x L] blocks of the mask.

    For each (q-chunk c, k-tile g) with any attention, restrict to the
    128-aligned q-subrange [off, off+L) of the chunk that actually attends
    (the sliding window gives each k-tile a ~640-wide q-band, so most blocks
    are narrower than QC — this cuts score/AV matmul streaming by ~38%).

    Returns (blocks, mask_buf): blocks[c] = list of (g, off, L, moff) with
    moff the column offset of the [128, L] mask slice in mask_buf (None if
    the block is all-keep); mask_buf is [128, total] float32.
    """
    mT = np.ascontiguousarray(mask2d.T)  # [k, q]
    n_chunks = s // QC
    n_ktiles = s // 128
    blocks = []
    cols = []
    tile_ids = {}
    total = 0
    for c in range(n_chunks):
        lst = []
        for g in range(n_ktiles):
            blk = mT[g * 128:(g + 1) * 128, c * QC:(c + 1) * QC]
            qs = np.flatnonzero(blk.any(axis=0))
            if qs.size == 0:
                continue
            off = (qs[0] // 128) * 128
            end = min(QC, ((qs[-1] // 128) + 1) * 128)
            sub = blk[:, off:end]
            if sub.all():
                lst.append((g, off, end - off, None))
            else:
                key = sub.tobytes()
                if key not in tile_ids:
                    tile_ids[key] = total
                    cols.append(sub.astype(np.float32))
                    total += sub.shape[1]
                lst.append((g, off, end - off, tile_ids[key]))
        assert lst, f"query chunk {c} attends to nothing"
        blocks.append(lst)
    if not cols:
        cols.append(np.zeros((128, 128), np.float32))
        total = 128
    return blocks, np.concatenate(cols, axis=1)


def _build_program(s, hid, blocks, mask_cols):
    """Emit the Bass/Tile program. Same program runs SPMD on all 8 cores."""
    import concourse.bacc as bacc
    import concourse.mybir as mybir
    import concourse.tile as tile
    from concourse import bass_isa

    dt = mybir.dt
    HT = hid // 128          # hidden contraction tiles (32)
    C = s // QC              # query chunks (4)
    PT = 4                   # wq piece size in t-tiles
    NP = HT // PT            # wq pieces per chunk (8)
    HB = 4                   # hid tiles per hT DMA batch

    nc = bacc.Bacc("TRN2", target_bir_lowering=False, debug=False,
                   num_devices=NCORES)

    # inputs are host-prepacked into SBUF-image layouts (partition-major) so
    # every DMA moves multi-KB contiguous runs per partition
    hT_d = nc.declare_dram_parameter("hT", [128, HT * s], dt.bfloat16, isOutput=False)
    wq_d = nc.declare_dram_parameter("wq", [128, HT * HQ * D], dt.bfloat16, isOutput=False)
    wk_d = nc.declare_dram_parameter("wk", [128, hid], dt.bfloat16, isOutput=False)
    wv_d = nc.declare_dram_parameter("wv", [128, hid], dt.bfloat16, isOutput=False)
    wo_d = nc.declare_dram_parameter("wo", [128, HQ * hid], dt.bfloat16, isOutput=False)
    cos_d = nc.declare_dram_parameter("cosT", [128, s], dt.bfloat16, isOutput=False)
    sin_d = nc.declare_dram_parameter("sinT", [128, s], dt.bfloat16, isOutput=False)
    msk_d = nc.declare_dram_parameter("masks", [128, mask_cols], dt.bfloat16, isOutput=False)
    eye_d = nc.declare_dram_parameter("eye", [128, 256], dt.bfloat16, isOutput=False)
    out_d = nc.declare_dram_parameter("out", [s, hid], dt.bfloat16, isOutput=True)

    with ExitStack() as ctx:
        tc = ctx.enter_context(tile.TileContext(nc))
        const = ctx.enter_context(tc.tile_pool(name="const", bufs=1))
        hpool = ctx.enter_context(tc.tile_pool(name="hpool", bufs=3))
        wqpool = ctx.enter_context(tc.tile_pool(name="wqpool", bufs=3))
        epool = ctx.enter_context(tc.tile_pool(name="epool", bufs=2))
        tpool = ctx.enter_context(tc.tile_pool(name="tpool", bufs=3))
        opool = ctx.enter_context(tc.tile_pool(name="opool", bufs=8))
        psum = ctx.enter_context(tc.tile_pool(name="psum", bufs=6, space="PSUM"))
        # at_ps accumulates across a whole head; in the shared rotation it
        # would stall the next head's first score alloc, so it gets own banks
        # at_ps + den alternate through this 2-bank pool; both are short-
        # lived relative to the main rotation and would stall sc allocs there
        psum_at = ctx.enter_context(tc.tile_pool(name="psum_at", bufs=2,
                                                 space="PSUM"))

        # ---- one-time loads ----
        # wk/wv resident (1MB each), wo resident (4.2MB, loaded after start),
        # wq streamed per chunk in pieces. hT double-buffered per chunk.
        wk_sb = const.tile([128, HT * D], dt.bfloat16, tag="wk")
        wv_sb = const.tile([128, HT * D], dt.bfloat16, tag="wv")
        eye_sb = const.tile([128, 256], dt.bfloat16, tag="eye")
        nc.gpsimd.dma_start(eye_sb[:], eye_d[:])  # [eye | ones]
        cos_sb = const.tile([128, s], dt.bfloat16, tag="cos")
        sin_sb = const.tile([128, s], dt.bfloat16, tag="sin")
        nc.gpsimd.dma_start(cos_sb[:], cos_d[:])
        nc.gpsimd.dma_start(sin_sb[:], sin_d[:])
        msk_sb = const.tile([128, mask_cols], dt.bfloat16, tag="msk")
        wo_sb = const.tile([128, HQ * hid], dt.bfloat16, tag="wo")

        # warm-up burst: dummy matmuls on a memset scratch keep the PE busy
        # while the first weights/activations stream in, so the HAM clock
        # gate reaches 8/8 before real work starts (else chunk 0 runs 1.2GHz)
        warm_sb = const.tile([128, QC], dt.bfloat16, tag="warm")
        nc.vector.memset(warm_sb[:], 0.0)
        warm_ps = psum.tile([128, QC], dt.float32, tag="ps", name="warm")
        for wi in range(16):
            nc.tensor.matmul(warm_ps[:], warm_sb[:, 0:128], warm_sb[:],
                             start=(wi == 0), stop=(wi == 15))

        # persistent per-chunk tensors
        q_sb = [[const.tile([128, QC], dt.bfloat16, tag=f"q{c}_{h}", name=f"q{c}_{h}")
                 for h in range(HQ)] for c in range(C)]
        kt_sb = [const.tile([128, QC], dt.bfloat16, tag=f"kt{c}", name=f"kt{c}")
                 for c in range(C)]
        v_sb = [[const.tile([128, 128], dt.bfloat16, tag=f"v{c}_{j}", name=f"v{c}_{j}")
                 for j in range(QC // 128)] for c in range(C)]
        atpool = ctx.enter_context(tc.tile_pool(name="atpool", bufs=8))
        at_t = {}            # (c, h) -> normalized attnT tile (rotating pool)

        # ---- phase 1: QKV projections (T layout) + RoPE + V transpose ----
        # Per chunk: pass A accumulates K/V (2 PSUM banks), pass B the 4 Q
        # heads (4 banks). hT chunk is SBUF-resident across both passes, so
        # only ~6 banks are ever live and boundaries never stall the PE.
        def rope_math(a, dest, c):
            # dest = a * cos + swap_halves(a) * sin   (all bf16, DVE 2x mode)
            cosc = cos_sb[:, c * QC:(c + 1) * QC]
            sinc = sin_sb[:, c * QC:(c + 1) * QC]
            b = tpool.tile([128, QC], dt.bfloat16, bufs=2, name="b")
            nc.gpsimd.dma_start(b[0:64, :], a[64:128, :])
            nc.gpsimd.dma_start(b[64:128, :], a[0:64, :])
            t1 = tpool.tile([128, QC], dt.bfloat16, bufs=2, name="t1")
            nc.vector.tensor_mul(t1[:], a[:], cosc)
            nc.vector.tensor_mul(b[:], b[:], sinc)
            nc.vector.tensor_add(dest[:], t1[:], b[:])

        for c in range(C):
            if c == 1 % C:
                nc.gpsimd.dma_start(msk_sb[:], msk_d[:])
            if c == 2 % C:
                for p in range(4):
                    q4 = HQ * hid // 4
                    nc.gpsimd.dma_start(wo_sb[:, p * q4:(p + 1) * q4],
                                        wo_d[:, p * q4:(p + 1) * q4])
            # hT chunk load: two half-chunk tiles (ring of 3), 4 batch DMAs
            # each. On chunk 0 the wk/wv pieces interleave with the hT batches
            # in PE consumption order so pass A never waits on a late weight.
            HH = HT // 2
            WP = HT // 8 * D
            halves = []
            for hf in range(2):
                htile = hpool.tile([128, HH * QC], dt.bfloat16, name="htc")
                for tb in range(HH // HB):
                    if c == 0:
                        p = hf * (HH // HB) + tb
                        nc.sync.dma_start(wk_sb[:, p * WP:(p + 1) * WP],
                                          wk_d[:, p * WP:(p + 1) * WP])
                        nc.sync.dma_start(wv_sb[:, p * WP:(p + 1) * WP],
                                          wv_d[:, p * WP:(p + 1) * WP])
                    base = (c * HT + hf * HH + tb * HB) * QC
                    nc.sync.dma_start(htile[:, tb * HB * QC:(tb + 1) * HB * QC],
                                      hT_d[:, base:base + HB * QC])
                halves.append(htile)

            def ht_at(t):
                return halves[t // HH][:, (t % HH) * QC:(t % HH + 1) * QC]
            # wq pieces for this chunk stream on the sync queue; piece i is
            # prefetched while piece i-1 computes (pass B below)
            wq_pc = []
            wq_q = nc.scalar
            for p in range(NP):
                wt = wqpool.tile([128, PT * HQ * D], dt.bfloat16, name="wqp")
                wq_q.dma_start(wt[:], wq_d[:, p * PT * HQ * D:(p + 1) * PT * HQ * D])
                wq_pc.append(wt)

            # pass A: K/V accumulation
            kt_ps = psum.tile([128, QC], dt.float32, tag="ps")
            vt_ps = psum.tile([128, QC], dt.float32, tag="ps")
            for t in range(HT):
                ht = ht_at(t)
                st, sp = (t == 0), (t == HT - 1)
                nc.tensor.matmul(kt_ps[:], wk_sb[:, t * D:(t + 1) * D], ht,
                                 start=st, stop=sp)
                nc.tensor.matmul(vt_ps[:], wv_sb[:, t * D:(t + 1) * D], ht,
                                 start=st, stop=sp)

            # drain K/V on scalar engine (fp32 PSUM -> bf16 SBUF)
            ka = tpool.tile([128, QC], dt.bfloat16, bufs=2, name="ka")
            nc.vector.tensor_copy(ka[:], kt_ps[:])
            vtT = epool.tile([128, QC], dt.bfloat16, bufs=2, name="vtT")
            nc.vector.tensor_copy(vtT[:], vt_ps[:])

            # pass B: 4 Q heads
            qt_ps = [psum.tile([128, QC], dt.float32, name=f"qt_ps{h}", tag="ps")
                     for h in range(HQ)]
            for t in range(HT):
                p, ts_ = t // PT, t % PT
                ht = ht_at(t)
                st, sp = (t == 0), (t == HT - 1)
                wt = wq_pc[p]
                for h in range(HQ):
                    nc.tensor.matmul(qt_ps[h][:],
                                     wt[:, (ts_ * HQ + h) * D:(ts_ * HQ + h + 1) * D],
                                     ht, start=st, stop=sp)

                # K rope + V transpose overlap the early Q accumulation
                if t == 0:
                    rope_math(ka, kt_sb[c], c)
                if t == 2:
                    for j in range(QC // 128):
                        tp = psum.tile([128, 128], dt.bfloat16, tag="ps")
                        nc.tensor.transpose(tp[:], vtT[:, j * 128:(j + 1) * 128],
                                            eye_sb[:, 0:128])
                        nc.vector.tensor_copy(v_sb[c][j][:], tp[:])
            for h in range(HQ):
                qa = tpool.tile([128, QC], dt.bfloat16, bufs=2, name=f"qa{h}")
                nc.vector.tensor_copy(qa[:], qt_ps[h][:])
                rope_math(qa, q_sb[c][h], c)

        # ---- phase 2: attention per (chunk, head) ----
        # Score matmuls run LOOK blocks ahead of the AV matmuls so the PE
        # never waits on the exp/mask chain; normalization of head h is
        # emitted during head h+1's stream, out-proj of chunk c during c+1.
        ones_sb = eye_sb[:, 128:256]  # [128,128] ones

        def emit_score(c, h, g, off, L, moff, ebuf, bi):
            # ragged block: only the 128-aligned q-subrange [off, off+L) of
            # the chunk attends to k-tile g (sliding window band)
            kc, j = g // (QC // 128), g % (QC // 128)
            sc = psum.tile([128, QC], dt.float32, tag="ps", name="sc")
            nc.tensor.matmul(sc[:, 0:L], kt_sb[kc][:, j * 128:(j + 1) * 128],
                             q_sb[c][h][:, off:off + L], start=True, stop=True)
            e = ebuf[:, bi * QC:bi * QC + L]
            nc.scalar.activation(e, sc[:, 0:L],
                                 mybir.ActivationFunctionType.Exp,
                                 scale=float(INV_NORM))
            if moff is not None:
                nc.vector.tensor_mul(e, e, msk_sb[:, moff:moff + L])
            return e

        def emit_norm(c, h, atu, esum, e_last, off_l, L_l):
            # den = ones^T @ (esum + e_last): partition-sum via two PE matmuls.
            # e_last skips the esum chain so den never waits on the DVE backlog
            den_ps = psum.tile([128, QC], dt.float32, tag="ps", name="den")
            nc.tensor.matmul(den_ps[:], ones_sb, esum[:], start=True, stop=False)
            nc.tensor.matmul(den_ps[:, off_l:off_l + L_l], ones_sb, e_last,
                             start=False, stop=True)
            recf = tpool.tile([128, QC], dt.float32, tag="recf", bufs=2, name="recf")
            nc.vector.reciprocal_approx_fast(out=recf[:], in_=den_ps[:])
            att = atpool.tile([128, QC], dt.bfloat16, name="at")
            nc.vector.tensor_mul(att[:], atu[:], recf[:])
            at_t[(c, h)] = att

        OC = 1024 if OPROJ_BF16_PSUM else QC
        o_dt = dt.bfloat16 if OPROJ_BF16_PSUM else dt.float32

        oq = []              # pending out-proj groups, drained 1 per step

        def emit_ogroup(c, oc, r):
            o_ps = psum.tile([128, OC], o_dt, tag="ps", name="o_ps")
            for h in range(HQ):
                nc.tensor.matmul(o_ps[:],
                                 at_t[(c, h)][:, r * 128:(r + 1) * 128],
                                 wo_sb[:, h * hid + oc * OC: h * hid + (oc + 1) * OC],
                                 start=(h == 0), stop=(h == HQ - 1))
            ob = opool.tile([128, OC], dt.bfloat16, name="ob")
            row = c * QC + r * 128
            if (oc + r) % 2 == 0:
                nc.vector.tensor_copy(ob[:], o_ps[:])
            else:
                nc.scalar.copy(ob[:], o_ps[:])
            dma_q = (nc.sync, nc.gpsimd)[(oc * 4 + r) % 2]
            dma_q.dma_start(out_d[row:row + 128, oc * OC:(oc + 1) * OC], ob[:])

        def emit_outproj(c):
            for oc in range(hid // OC):
                for r in range(QC // 128):
                    oq.append((c, oc, r))

        esum_eng = nc.gpsimd if ESUM_ON_GPSIMD else nc.vector

        stream = []          # (c, h, i, n, g, off, L, moff)
        for c in range(C):
            for h in range(HQ):
                blks = blocks[c]
                for i, (g, off, L, moff) in enumerate(blks):
                    stream.append((c, h, i, len(blks), g, off, L, moff))
        N = len(stream)
        es = {}
        ebufs = {}
        pend = None          # (c, h, atu, esum, e_last, off, L) pending norm
        oc_pend = None       # (c, h) whose norm just fired; outproj pends
        pf = 0               # prefetch pointer

        at_ps = None
        esum = None
        for i in range(N):
            c, h, bi, n, g, off, L, moff = stream[i]
            while pf < N and pf < i + LOOK + 1:
                if pf <= i:
                    pf = i
                cc, hh, bb, nn, gg, oo, ll, mm = stream[pf]
                if bb == 0:
                    ebufs[(cc, hh)] = epool.tile([128, 8 * QC], dt.bfloat16,
                                                 name="eb")
                es[pf] = emit_score(cc, hh, gg, oo, ll, mm,
                                    ebufs[(cc, hh)], bb)
                pf += 1
            if bi == 0:
                at_ps = psum_at.tile([128, QC], dt.float32, tag="at", name="at_ps")
                esum = tpool.tile([128, QC], dt.bfloat16, tag="esum", bufs=2,
                                  name="esum")
                nc.vector.memset(esum[:], 0.0)
            kc, j = g // (QC // 128), g % (QC // 128)
            st, sp = (bi == 0), (bi == n - 1)
            e = es.pop(i)
            nc.tensor.matmul(at_ps[:, off:off + L], v_sb[kc][j][:], e,
                             start=st, stop=sp)
            if bi < n - 1:
                esum_eng.tensor_add(esum[:, off:off + L],
                                    esum[:, off:off + L], e)
            if oq:
                emit_ogroup(*oq.pop(0))
            if bi == min(2, n - 1) and pend is not None:
                emit_norm(*pend)
                oc_pend = (pend[0], pend[1])
                pend = None
            if bi == min(6, n - 2) and oc_pend is not None:
                if oc_pend[1] == HQ - 1:
                    emit_outproj(oc_pend[0])
                oc_pend = None
            if bi == n - 1:
                # drain the unnormalized attnT on the scalar engine
                atu = tpool.tile([128, QC], dt.bfloat16, tag="atu", bufs=2,
                                 name="atu")
                nc.scalar.copy(atu[:], at_ps[:])
                ebufs.pop((c, h), None)
                assert pend is None
                pend = (c, h, atu, esum, e, off, L)
        emit_norm(*pend)
        emit_outproj(C - 1)
        while oq:
            emit_ogroup(*oq.pop(0))

    nc.compile()
    return nc


def _prep_inputs(hidden_states, attention_mask, Wq, Wk, Wv, Wo):
    """Host-side sharding + layout prep. Returns (in_maps, blocks, n_mask, s, hid)."""
    hs = np.asarray(hidden_states)
    assert hs.shape[0] == 1, "kernel assumes batch 1"
    s, hid = hs.shape[1], hs.shape[2]
    mask = np.asarray(attention_mask)[0]
    Wq = np.asarray(Wq); Wk = np.asarray(Wk); Wv = np.asarray(Wv); Wo = np.asarray(Wo)

    # SBUF-image packing: x[(t p), c] -> [p, (t c)] so DMAs are contiguous
    def pack(w, tiles):
        return np.ascontiguousarray(
            w.reshape(tiles, 128, -1).transpose(1, 0, 2).reshape(128, -1)
        ).astype(BF16)

    hTn = np.asarray(hs[0].T).reshape(hid // 128, 128, s // QC, QC)
    hT = np.ascontiguousarray(hTn.transpose(1, 2, 0, 3).reshape(128, -1)).astype(BF16)
    # layout: hT[p, ((c * HT + t) * QC + q)]
    cosT, sinT = _rope_tables(s)
    blocks, mask_buf = _classify_mask(mask, s)
    eye_ones = np.concatenate(
        [np.eye(128, dtype=np.float32), np.ones((128, 128), np.float32)],
        axis=1).astype(BF16)

    masks_pk = np.ascontiguousarray(mask_buf).astype(BF16)
    mask_cols = masks_pk.shape[1]

    in_maps = []
    for i in range(NCORES):
        wq_i = pack(Wq[:, i * HQ:(i + 1) * HQ, :].reshape(hid, HQ * D), hid // 128)
        wk_i = pack(Wk[:, i, :], hid // 128)
        wv_i = pack(Wv[:, i, :], hid // 128)
        wo_i = pack(Wo[i * HQ:(i + 1) * HQ].reshape(HQ * D, hid), HQ)
        in_maps.append({
            "hT": hT, "wq": wq_i, "wk": wk_i, "wv": wv_i, "wo": wo_i,
            "cosT": cosT, "sinT": sinT, "masks": masks_pk, "eye": eye_ones,
        })
    return in_maps, blocks, mask_cols, s, hid


def _run(hidden_states, attention_mask, Wq, Wk, Wv, Wo, trace=False):
    from concourse.bass_utils import run_bass_kernel_spmd

    in_maps, blocks, mask_cols, s, hid = _prep_inputs(
        hidden_states, attention_mask, Wq, Wk, Wv, Wo)
    nc = _build_program(s, hid, blocks, mask_cols)
    res = run_bass_kernel_spmd(nc, in_maps, core_ids=list(range(NCORES)),
                               trace=trace)
    out = np.zeros((s, hid), np.float32)
    for i in range(NCORES):
        out += res.results[i]["out"].astype(np.float32)
    return out[None, :, :], res


def kernel(hidden_states, attention_mask, Wq, Wk, Wv, Wo):
    out, _ = _run(hidden_states, attention_mask, Wq, Wk, Wv, Wo, trace=False)
    return out
